# revision 11
# baseline (speedup 1.0000x reference)
"""Trainium2 Bass kernel for multi-head attention (B=4, N=2048, C=768, H=12).

Sharding: 8 cores = 4 batches x 2 head-halves. Each core computes Q/K/V and
attention for its 6 heads (3 head-pairs) over the full 2048-token sequence,
then the final projection restricted to its 384 feature columns, producing a
partial [2048, 768] output (f16). The host sums the two partials per batch
(the even core folds in the bias). No duplicated projection work, no
collectives.

All matmul operands are fp16, accumulation f32 in PSUM, softmax norm in f32.

The kernel is paced by ScalarE exp (~25M exps/core, [128,1024] per ACTIVATE =
~1147ns each). Everything else hides under the exp stream:
- startup: inputs stream on the gpsimd DMA queue in need-order (pair-0 K/Q
  weights, x chunk 0, ...); dummy matmuls on constants warm the PE HAM clock
  while the first DMAs land, so the first real matmuls run at 2.4 GHz.
- blocks interleave pairs (0,*),(1,0),(2,0),(1,1),(2,1),... so the output
  projection (which needs all three pairs' outT for a token range) spreads
  over two blocks of PE slack instead of piling up at the end.
- scores are software-pipelined one key-tile ahead of the exp that consumes
  them; AV accumulates v'=[v|ones] so row 64 carries the softmax denominator.
- normalization: 1/den via reciprocal_approx_fast (5x faster than the exact
  DVE op), broadcast across partitions with gpsimd.partition_broadcast (no
  PE outer product, no PSUM bank: the old PE broadcast shared the po banks
  with the live accumulators and pushed every fin a full block late).
"""

import numpy as np

B, N, C = 4, 2048, 768
H, HD = 12, 64
SCALE = HD ** -0.5
P = 128
CT = C // P          # 6 contraction tiles for QKV projections
HC = C // 2          # 384 feature columns per core
PCT = HC // P        # 3 contraction tiles for the final projection
PAIRS = 3            # head pairs per core
JT = N // P          # 16 key tiles
IB = N // 512        # 4 query blocks
TKB = 512            # token-block width of projection matmuls
NCORES = 8

_cache = {}


def _build_bass():
    import concourse.bass as bass
    import concourse.tile as tile
    import concourse.mybir as mybir
    from concourse import bacc
    from concourse.bass import ts, ds
    from contextlib import ExitStack

    f32 = mybir.dt.float32
    f16 = mybir.dt.float16
    Exp = mybir.ActivationFunctionType.Exp

    nc = bacc.Bacc("TRN2", target_bir_lowering=False, debug=False)

    # all inputs pre-swizzled on the host to partition-major layouts so
    # every load is a fully contiguous DMA; wq/wk additionally pair-major
    # so pair 0 can load first
    xt_d = nc.dram_tensor("xt", [P, N * CT], f16, kind="ExternalInput").ap()
    wq_d = nc.dram_tensor("wq", [P, PAIRS * CT * P], f16, kind="ExternalInput").ap()
    wk_d = nc.dram_tensor("wk", [P, PAIRS * CT * P], f16, kind="ExternalInput").ap()
    wv_d = nc.dram_tensor("wv", [P, CT * HC], f16, kind="ExternalInput").ap()
    wp_d = nc.dram_tensor("wp", [P, PCT * C], f16, kind="ExternalInput").ap()
    bb_d = nc.dram_tensor("bb", [P, C], f32, kind="ExternalInput").ap()
    out_d = nc.dram_tensor("out", [N, C], f16, kind="ExternalOutput").ap()

    xt_r = xt_d.rearrange("p (t o n) -> p t o n", t=N // TKB, o=CT)
    wq_r = wq_d.rearrange("p (h o n) -> p h o n", h=PAIRS, o=CT)
    wk_r = wk_d.rearrange("p (h o n) -> p h o n", h=PAIRS, o=CT)
    wv_r = wv_d.rearrange("p (o n) -> p o n", o=CT)
    wp_r = wp_d.rearrange("p (o n) -> p o n", o=PCT)
    out_r = out_d.rearrange("(t p) n -> t p n", p=P)

    with tile.TileContext(nc) as tc:
        with ExitStack() as ctx:
            persist = ctx.enter_context(tc.tile_pool(name="persist", bufs=1))
            xt_sb = persist.tile([P, N // TKB, CT, TKB], f16, name="xt_sb")
            wk_sb = persist.tile([P, PAIRS, CT, P], f16, name="wk_sb")
            wq_sb = persist.tile([P, PAIRS, CT, P], f16, name="wq_sb")
            wv_sb = persist.tile([P, CT, HC], f16, name="wv_sb")
            wp_sb = persist.tile([P, PCT, C], f16, name="wp_sb")
            bias_sb = persist.tile([P, C], f32, name="bias_sb")

            # input DMAs in need-order on the gpsimd queue (that engine's
            # preamble finishes ~1us before sync's, so first bytes land
            # earlier); den/out DMAs use the sync queue instead.
            nc.gpsimd.dma_start(wk_sb[:, 0], wk_r[:, 0])
            nc.gpsimd.dma_start(wq_sb[:, 0], wq_r[:, 0])
            # x chunk 0 split in two so the first kq matmuls start earlier
            nc.gpsimd.dma_start(xt_sb[:, 0, 0:3], xt_r[:, 0, 0:3])
            nc.gpsimd.dma_start(xt_sb[:, 0, 3:6], xt_r[:, 0, 3:6])
            nc.gpsimd.dma_start(xt_sb[:, 1], xt_r[:, 1])
            nc.gpsimd.dma_start(wv_sb[:], wv_r)
            nc.gpsimd.dma_start(xt_sb[:, 2], xt_r[:, 2])
            nc.gpsimd.dma_start(wk_sb[:, 1:3], wk_r[:, 1:3])
            nc.gpsimd.dma_start(xt_sb[:, 3], xt_r[:, 3])
            nc.gpsimd.dma_start(wq_sb[:, 1:3], wq_r[:, 1:3])
            nc.gpsimd.dma_start(wp_sb[:], wp_r)
            nc.gpsimd.dma_start(bias_sb[:], bb_d)

            # pair-packed K/Q: partitions 0:64 even head, 64:128 odd head
            kT_sb = persist.tile([P, PAIRS, N], f16, name="kT_sb")
            qT_sb = persist.tile([P, PAIRS, N], f16, name="qT_sb")
            # V + ones column: [keys 128, key-tile, head, 66] (col 64 = ones)
            v_all = persist.tile([P, JT, 6, 66], f16, name="v_all")
            outT_sb = persist.tile([P, PAIRS, N], f16, name="outT_sb")
            ones_sb = persist.tile([33, 64], f16, name="ones_sb")
            warm_row = persist.tile([1, TKB], f16, name="warm_row")
            # rows 0/32 receive each pair's two denominators; rows 1-31 only
            # feed wasted reciprocal lanes but must be nonzero and initialized
            den_q = persist.tile([33, 512], f32, name="den_q")
            with nc.allow_low_precision(reason="ones constant is exact in f16"):
                nc.vector.tensor_copy(
                    v_all[:, :, :, 64:66],
                    nc.const_aps.tensor(1.0, [P, JT, 6, 2], f32),
                )
                nc.vector.tensor_copy(
                    ones_sb[:], nc.const_aps.tensor(1.0, [33, 64], f32)
                )
                nc.vector.tensor_copy(
                    warm_row[:], nc.const_aps.tensor(1.0, [1, TKB], f32)
                )
            nc.vector.tensor_copy(
                den_q[:], nc.const_aps.tensor(1.0, [33, 512], f32)
            )
            # dummy exp forces the ACT table load off the critical path
            warm_sb = persist.tile([1, 64], f16, name="warm_sb")
            nc.scalar.activation(warm_sb[:], ones_sb[0:1, :], Exp)

            apsum = ctx.enter_context(
                tc.tile_pool(name="apsum", bufs=2, space="PSUM")
            )
            spsum = ctx.enter_context(
                tc.tile_pool(name="spsum", bufs=2, space="PSUM")
            )
            opsum = ctx.enter_context(
                tc.tile_pool(name="opsum", bufs=2, space="PSUM")
            )
            expt_pool = ctx.enter_context(tc.tile_pool(name="expt", bufs=6))
            nrm_pool = ctx.enter_context(tc.tile_pool(name="nrm", bufs=2))
            poS_pool = ctx.enter_context(tc.tile_pool(name="poSp", bufs=4))
            outsb_pool = ctx.enter_context(tc.tile_pool(name="outsb", bufs=6))

            # ~6us of dummy matmuls on constants while input DMAs stream:
            # keeps the PE HAM window busy so real matmuls start at 2.4 GHz
            warm_ps = apsum.tile([P, TKB], f32, tag="aps", name="warm_ps")
            for _ in range(12):
                nc.tensor.matmul(
                    warm_ps[0:64, :],
                    ones_sb[0:1, 0:64],
                    warm_row[0:1, :],
                    start=True,
                    stop=True,
                )

            def kq_group(p, tb, which):
                w_sb, dst = (
                    (wk_sb, kT_sb) if which == "k" else (wq_sb, qT_sb)
                )
                ps = apsum.tile([P, TKB], f32, tag="aps")
                for c in range(CT):
                    nc.tensor.matmul(
                        ps[:],
                        w_sb[:, p, c, :],
                        xt_sb[:, tb, c, :],
                        start=(c == 0),
                        stop=(c == CT - 1),
                    )
                with nc.allow_low_precision(reason="f16 kq path"):
                    nc.vector.tensor_copy(dst[:, p, ts(tb, TKB)], ps[:])

            def kq_thunks(p, order="kq"):
                # K first: a pair's scores need all of K but only the
                # current query block of Q
                return [
                    (lambda p=p, tb=tb, w=w: kq_group(p, tb, w))
                    for w in order
                    for tb in range(N // TKB)
                ]

            def v_tile(tt):
                # V projection, all 6 heads at once for one 128-token tile:
                # token-tile stationary, wv moving
                ps = apsum.tile([P, TKB], f32, tag="aps")
                for c in range(CT):
                    nc.tensor.matmul(
                        ps[:, 0:HC],
                        xt_sb[:, tt // 4, c, ts(tt % 4, P)],
                        wv_sb[:, c, :],
                        start=(c == 0),
                        stop=(c == CT - 1),
                    )
                with nc.allow_low_precision(reason="f16 value path"):
                    nc.vector.tensor_copy(
                        v_all[:, tt, :, 0:64],
                        ps[:, 0:HC].rearrange("p (h e) -> p h e", e=64),
                    )

            def out_chunk(git, ci, ob, pp=None, skip_mm=False):
                # half of the final projection for one 128-token tile
                n0, n1 = ((0, 512), (512, 768))[ci]
                if pp is None:
                    pp = apsum.tile([P, TKB], f32, tag="aps")
                if not skip_mm:
                    for t in range(PAIRS):
                        nc.tensor.matmul(
                            pp[:, 0 : n1 - n0],
                            outT_sb[:, t, ds(git * P, P)],
                            wp_sb[:, t, n0:n1],
                            start=(t == 0),
                            stop=(t == PAIRS - 1),
                        )
                with nc.allow_low_precision(reason="f16 partial output"):
                    nc.vector.tensor_add(
                        ob[:, n0:n1], pp[:, 0 : n1 - n0], bias_sb[:, n0:n1]
                    )
                if ci == 1:
                    nc.sync.dma_start(out_r[git], ob[:])

            def og_thunks(gits):
                th = []
                for g in gits:
                    ob_box = []

                    def c0(g=g, ob_box=ob_box):
                        ob_box.append(
                            outsb_pool.tile([P, C], f16, tag="ob", name="ob")
                        )
                        out_chunk(g, 0, ob_box[0])

                    def c1(g=g, ob_box=ob_box):
                        out_chunk(g, 1, ob_box[0])

                    th += [c0, c1]
                return th

            def attention(p, ib, with_v=False, bg=None, fin=None, last=False):
                po0 = opsum.tile([P, 512], f32, tag="po")
                po1 = opsum.tile([P, 512], f32, tag="po")
                pos = (po0, po1)

                def scores(jt):
                    ss = spsum.tile([P, 1024], f32, tag="ss")
                    nc.tensor.matmul(
                        ss[:, 0:512],
                        kT_sb[0:64, p, ts(jt, P)],
                        qT_sb[0:64, p, ts(ib, 512)],
                        start=True,
                        stop=True,
                    )
                    nc.tensor.matmul(
                        ss[:, 512:1024],
                        kT_sb[64:128, p, ts(jt, P)],
                        qT_sb[64:128, p, ts(ib, 512)],
                        start=True,
                        stop=True,
                    )
                    return ss

                ss_cur = scores(0)
                if with_v:
                    v_tile(0)
                for jt in range(JT):
                    et = expt_pool.tile([P, 1024], f16, tag="et")
                    nc.scalar.activation(et[:], ss_cur[:], Exp, scale=SCALE)
                    # next key-tile's scores run on the PE while the exp
                    # above works through this one
                    if jt < JT - 1:
                        ss_cur = scores(jt + 1)
                        if with_v:
                            v_tile(jt + 1)
                    if bg is not None and jt < len(bg) and bg[jt] is not None:
                        bg[jt]()
                    if jt == 6 and fin is not None:
                        fin()
                    for hh in range(2):
                        nc.tensor.matmul(
                            pos[hh][0:65, :],
                            v_all[:, jt, 2 * p + hh, 0:65],
                            et[:, hh * 512 : (hh + 1) * 512],
                            start=(jt == 0),
                            stop=(jt == JT - 1),
                        )
                # denominators straight from PSUM row 64 via small DVE
                # copies ahead of the big poS copies (keeps the reciprocal
                # off the copy tail; DMA cannot read PSUM)
                for hh in range(2):
                    nc.vector.tensor_copy(
                        den_q[32 * hh : 32 * hh + 1, :], pos[hh][64:65, :]
                    )
                poS_all = []
                for hh in range(2):
                    poS = poS_pool.tile([64, 512], f32, tag="poS")
                    nc.vector.tensor_copy(poS[:], pos[hh][0:64, :])
                    poS_all.append(poS)
                rd32 = nrm_pool.tile([33, 512], f32, tag="rd32")
                nc.vector.reciprocal_approx_fast(rd32[:], den_q[:])
                rd_q = nrm_pool.tile([33, 512], f16, tag="rd_q")
                with nc.allow_low_precision(reason="f16 reciprocal scale"):
                    nc.vector.tensor_copy(rd_q[:], rd32[:])

                def fin_thunk():
                    rbs = []
                    for hh in range(2):
                        # partition-broadcast 1/den via PE outer product into
                        # a short-lived apsum slot (the po banks are occupied
                        # by live accumulators, which would delay this a
                        # whole block). The final block's fin runs after the
                        # tail prefill has filled every aps/ss slot with work
                        # that depends on this fin's output, so it must use
                        # the po banks instead (free once the casts are done).
                        rb_ps = (opsum if last else apsum).tile(
                            [P, 512], f32, tag="po" if last else "aps"
                        )
                        nc.tensor.matmul(
                            rb_ps[0:64, :],
                            ones_sb[32 * hh : 32 * hh + 1, 0:64],
                            rd_q[32 * hh : 32 * hh + 1, :],
                            start=True,
                            stop=True,
                        )
                        rbs.append(rb_ps)
                    with nc.allow_low_precision(reason="f16 attn output"):
                        if last:
                            # per-git muls so the tail projections can start
                            # on the first token tile while the rest finish
                            for g in range(4):
                                for hh in range(2):
                                    nc.vector.tensor_mul(
                                        outT_sb[
                                            hh * 64 : (hh + 1) * 64,
                                            p,
                                            ds(ib * 512 + g * P, P),
                                        ],
                                        poS_all[hh][:, ds(g * P, P)],
                                        rbs[hh][0:64, ds(g * P, P)],
                                    )
                        else:
                            for hh in range(2):
                                nc.vector.tensor_mul(
                                    outT_sb[hh * 64 : (hh + 1) * 64, p, ts(ib, 512)],
                                    poS_all[hh][:],
                                    rbs[hh][0:64, :],
                                )

                return fin_thunk

            def slots(d, njt=JT):
                out = [None] * njt
                for k, th in d.items():
                    if isinstance(th, list):
                        def chain(ths=th):
                            for t in ths:
                                t()
                        out[k] = chain
                    else:
                        out[k] = th
                return out

            kq0 = kq_thunks(0)   # [k0,k1,k2,k3, q0,q1,q2,q3]
            kq1 = kq_thunks(1)
            kq2 = kq_thunks(2)
            og = og_thunks(range(12))   # 24 chunk thunks for gits 0..11

            kq0[0]()            # k tb0
            kq0[4]()            # q tb0
            # each pair's first block carries that pair's remaining K
            # projections (scores consume kT tile-by-tile); q projections
            # land one block before their query block needs them
            fin = attention(0, 0, with_v=True, bg=slots({
                1: kq0[1], 4: kq0[2], 7: kq0[3], 10: kq0[5],
            }))
            fin = attention(0, 1, bg=slots({
                0: kq0[6], 3: kq1[0], 8: kq1[4], 12: kq1[1],
            }), fin=fin)
            fin = attention(0, 2, bg=slots({
                0: kq0[7], 3: kq1[2], 7: kq1[3], 11: kq1[5],
            }), fin=fin)
            fin = attention(0, 3, bg=slots({
                0: kq1[6], 3: kq2[0], 7: kq2[4], 11: kq2[1],
            }), fin=fin)
            fin = attention(1, 0, bg=slots({
                1: kq2[2], 5: kq2[3], 9: kq1[7], 13: kq2[5],
            }), fin=fin)
            fin = attention(2, 0, bg=slots({
                2: kq2[6], 6: kq2[7],
            }), fin=fin)
            # og(gits of ib) needs outT of all three pairs for that ib: the
            # last fin to land is fin(2, ib), fired at jt6 of the following
            # block, so each og batch spreads over the next two blocks
            fin = attention(1, 1, bg=slots({
                8: og[0], 9: og[1], 11: og[2], 12: og[3],
            }), fin=fin)
            fin = attention(2, 1, bg=slots({
                0: og[4], 2: og[5], 4: og[6], 6: og[7],
            }), fin=fin)
            fin = attention(1, 2, bg=slots({
                8: og[8], 9: og[9], 11: og[10], 12: og[11],
            }), fin=fin)
            fin = attention(2, 2, bg=slots({
                0: og[12], 2: og[13], 4: og[14], 6: og[15],
            }), fin=fin)
            fin = attention(1, 3, bg=slots({
                8: og[16], 9: og[17], 11: og[18], 12: og[19],
            }), fin=fin)
            fin = attention(2, 3, bg=slots({
                0: og[20], 2: og[21], 4: og[22], 6: og[23],
            }), fin=fin, last=True)

            # tail: prefill the pair-0/1 contributions for gits 12-14 while
            # the last exps and the final normalization chain run, then
            # finish with the pair-2 matmuls, bias adds and output DMAs.
            sst0 = spsum.tile([P, 1024], f32, tag="ss", name="sst0")
            sst1 = spsum.tile([P, 1024], f32, tag="ss", name="sst1")
            tail_slots = [
                apsum.tile([P, TKB], f32, tag="aps", name="tp0"),
                apsum.tile([P, TKB], f32, tag="aps", name="tp1"),
                sst0[:, 0:512],
                sst0[:, 512:1024],
                sst1[:, 0:512],
                sst1[:, 512:1024],
            ]
            tail_gc = [(g, ci) for g in range(12, 15) for ci in (0, 1)]
            for k, (g, ci) in enumerate(tail_gc):
                n0, n1 = ((0, 512), (512, 768))[ci]
                pp = tail_slots[k]
                for t in (0, 1):
                    nc.tensor.matmul(
                        pp[:, 0 : n1 - n0],
                        outT_sb[:, t, ds(g * P, P)],
                        wp_sb[:, t, n0:n1],
                        start=(t == 0),
                        stop=False,
                    )
            fin()                                   # outT(2, 3)
            obs = [
                outsb_pool.tile([P, C], f16, tag="ob", name=f"obt{g}")
                for g in range(4)
            ]
            for k, (g, ci) in enumerate(tail_gc):
                n0, n1 = ((0, 512), (512, 768))[ci]
                pp = tail_slots[k]
                nc.tensor.matmul(
                    pp[:, 0 : n1 - n0],
                    outT_sb[:, 2, ds(g * P, P)],
                    wp_sb[:, 2, n0:n1],
                    start=False,
                    stop=True,
                )
                out_chunk(g, ci, obs[g - 12], pp=pp, skip_mm=True)
            # git 15: full projection at the very end
            for ci in (0, 1):
                out_chunk(15, ci, obs[3])

    nc.compile()
    return nc


def _get_nc():
    if "nc" not in _cache:
        _cache["nc"] = _build_bass()
    return _cache["nc"]


def _prep_in_maps(x, w_qkv, w_proj, b_proj):
    x = np.asarray(x, np.float32)
    w_qkv = np.asarray(w_qkv, np.float32)
    w_proj = np.asarray(w_proj, np.float32)
    b_proj = np.asarray(b_proj, np.float32)

    def swz(w):
        # [C_in, M] -> partition-major [128, CT_in * M] (contiguous DMA)
        ct, m = w.shape[0] // P, w.shape[1]
        return np.ascontiguousarray(
            w.reshape(ct, P, m).transpose(1, 0, 2).reshape(P, ct * m)
        ).astype(np.float16)

    def swz_kq(w):
        # [C_in=768, 384] -> pair-major [128, PAIRS * CT * 128]
        return np.ascontiguousarray(
            w.reshape(CT, P, PAIRS, P).transpose(1, 2, 0, 3).reshape(P, -1)
        ).astype(np.float16)

    wq = np.ascontiguousarray(w_qkv[0:C].T)
    wk = np.ascontiguousarray(w_qkv[C : 2 * C].T)
    wv = np.ascontiguousarray(w_qkv[2 * C : 3 * C].T)
    wp = w_proj.T
    bb = np.ascontiguousarray(np.broadcast_to(b_proj[None, :], (P, C)))
    zb = np.zeros((P, C), np.float32)

    in_maps = []
    for core in range(NCORES):
        b, half = core // 2, core % 2
        # x[b].T [C, N] -> [128, TB, CT, 512] partition-major, contiguous
        xt = np.ascontiguousarray(
            x[b].T.reshape(CT, P, N // TKB, TKB)
            .transpose(1, 2, 0, 3)
            .reshape(P, -1)
        ).astype(np.float16)
        sl = slice(half * HC, (half + 1) * HC)
        in_maps.append(
            {
                "xt": xt,
                "wq": swz_kq(wq[:, sl]),
                "wk": swz_kq(wk[:, sl]),
                "wv": swz(wv[:, sl]),
                "wp": swz(wp[sl, :]),
                "bb": bb if half == 0 else zb,
            }
        )
    return in_maps


def run(x, w_qkv, w_proj, b_proj, trace=False):
    from concourse import bass_utils

    nc = _get_nc()
    in_maps = _prep_in_maps(x, w_qkv, w_proj, b_proj)
    br = bass_utils.run_bass_kernel_spmd(
        nc, in_maps, core_ids=list(range(NCORES)), trace=trace
    )
    y = np.empty((B, N, C), np.float32)
    for b in range(B):
        y[b] = np.asarray(br.results[2 * b]["out"], np.float32)
        y[b] += np.asarray(br.results[2 * b + 1]["out"], np.float32)
    return y, br


def kernel(x, w_qkv, w_proj, b_proj):
    y, _ = run(x, w_qkv, w_proj, b_proj, trace=False)
    return y


# revision 14
# speedup vs baseline: 1.0142x; 1.0142x over previous
"""Trainium2 Bass kernel for multi-head attention (B=4, N=2048, C=768, H=12).

Sharding: 8 cores = 4 batches x 2 head-halves. Each core computes Q/K/V and
attention for its 6 heads (3 head-pairs) over the full 2048-token sequence,
then the final projection restricted to its 384 feature columns, producing a
partial [2048, 768] output (f16). The host sums the two partials per batch
(the even core folds in the bias). No duplicated projection work, no
collectives.

All matmul operands are fp16, accumulation f32 in PSUM, softmax norm in f32.

The kernel is paced by ScalarE exp (~25M exps/core, [128,1024] per ACTIVATE =
~1147ns each). Everything else hides under the exp stream:
- startup: inputs stream on the gpsimd DMA queue in need-order (pair-0 K/Q
  weights, x chunk 0, ...); dummy matmuls on constants warm the PE HAM clock
  while the first DMAs land, so the first real matmuls run at 2.4 GHz.
- blocks interleave pairs (0,*),(1,0),(2,0),(1,1),(2,1),... so the output
  projection (which needs all three pairs' outT for a token range) spreads
  over two blocks of PE slack instead of piling up at the end.
- scores are software-pipelined one key-tile ahead of the exp that consumes
  them; AV accumulates v'=[v|ones] so row 64 carries the softmax denominator.
- normalization: 1/den via reciprocal_approx_fast (5x faster than the exact
  DVE op), broadcast across partitions with gpsimd.partition_broadcast (no
  PE outer product, no PSUM bank: the old PE broadcast shared the po banks
  with the live accumulators and pushed every fin a full block late).
"""

import numpy as np

B, N, C = 4, 2048, 768
H, HD = 12, 64
SCALE = HD ** -0.5
P = 128
CT = C // P          # 6 contraction tiles for QKV projections
HC = C // 2          # 384 feature columns per core
PCT = HC // P        # 3 contraction tiles for the final projection
PAIRS = 3            # head pairs per core
JT = N // P          # 16 key tiles
IB = N // 512        # 4 query blocks
TKB = 512            # token-block width of projection matmuls
NCORES = 8

_cache = {}


def _build_bass():
    import concourse.bass as bass
    import concourse.tile as tile
    import concourse.mybir as mybir
    from concourse import bacc
    from concourse.bass import ts, ds
    from contextlib import ExitStack

    f32 = mybir.dt.float32
    f16 = mybir.dt.float16
    Exp = mybir.ActivationFunctionType.Exp

    nc = bacc.Bacc("TRN2", target_bir_lowering=False, debug=False)

    # all inputs pre-swizzled on the host to partition-major layouts so
    # every load is a fully contiguous DMA; wq/wk additionally pair-major
    # so pair 0 can load first
    xt_d = nc.dram_tensor("xt", [P, N * CT], f16, kind="ExternalInput").ap()
    wq_d = nc.dram_tensor("wq", [P, PAIRS * CT * P], f16, kind="ExternalInput").ap()
    wk_d = nc.dram_tensor("wk", [P, PAIRS * CT * P], f16, kind="ExternalInput").ap()
    wv_d = nc.dram_tensor("wv", [P, CT * HC], f16, kind="ExternalInput").ap()
    wp_d = nc.dram_tensor("wp", [P, PCT * C], f16, kind="ExternalInput").ap()
    bb_d = nc.dram_tensor("bb", [P, C], f32, kind="ExternalInput").ap()
    out_d = nc.dram_tensor("out", [N, C], f16, kind="ExternalOutput").ap()

    xt_r = xt_d.rearrange("p (t o n) -> p t o n", t=N // TKB, o=CT)
    wq_r = wq_d.rearrange("p (h o n) -> p h o n", h=PAIRS, o=CT)
    wk_r = wk_d.rearrange("p (h o n) -> p h o n", h=PAIRS, o=CT)
    wv_r = wv_d.rearrange("p (o n) -> p o n", o=CT)
    wp_r = wp_d.rearrange("p (o n) -> p o n", o=PCT)
    out_r = out_d.rearrange("(t p) n -> t p n", p=P)

    with tile.TileContext(nc) as tc:
        with ExitStack() as ctx:
            persist = ctx.enter_context(tc.tile_pool(name="persist", bufs=1))
            xt_sb = persist.tile([P, N // TKB, CT, TKB], f16, name="xt_sb")
            wk_sb = persist.tile([P, PAIRS, CT, P], f16, name="wk_sb")
            wq_sb = persist.tile([P, PAIRS, CT, P], f16, name="wq_sb")
            wv_sb = persist.tile([P, CT, HC], f16, name="wv_sb")
            wp_sb = persist.tile([P, PCT, C], f16, name="wp_sb")
            bias_sb = persist.tile([P, C], f32, name="bias_sb")

            # input DMAs in need-order on the gpsimd queue (that engine's
            # preamble finishes ~1us before sync's, so first bytes land
            # earlier); den/out DMAs use the sync queue instead.
            nc.gpsimd.dma_start(wk_sb[:, 0], wk_r[:, 0])
            nc.gpsimd.dma_start(wq_sb[:, 0], wq_r[:, 0])
            # x chunk 0 split in two so the first kq matmuls start earlier
            nc.gpsimd.dma_start(xt_sb[:, 0, 0:3], xt_r[:, 0, 0:3])
            nc.gpsimd.dma_start(xt_sb[:, 0, 3:6], xt_r[:, 0, 3:6])
            nc.gpsimd.dma_start(xt_sb[:, 1], xt_r[:, 1])
            nc.gpsimd.dma_start(wv_sb[:], wv_r)
            nc.gpsimd.dma_start(xt_sb[:, 2], xt_r[:, 2])
            nc.gpsimd.dma_start(wk_sb[:, 1:3], wk_r[:, 1:3])
            nc.gpsimd.dma_start(xt_sb[:, 3], xt_r[:, 3])
            nc.gpsimd.dma_start(wq_sb[:, 1:3], wq_r[:, 1:3])
            nc.gpsimd.dma_start(wp_sb[:], wp_r)
            nc.gpsimd.dma_start(bias_sb[:], bb_d)

            # pair-packed K/Q: partitions 0:64 even head, 64:128 odd head
            kT_sb = persist.tile([P, PAIRS, N], f16, name="kT_sb")
            qT_sb = persist.tile([P, PAIRS, N], f16, name="qT_sb")
            # V + ones column: [keys 128, key-tile, head, 66] (col 64 = ones)
            v_all = persist.tile([P, JT, 6, 66], f16, name="v_all")
            outT_sb = persist.tile([P, PAIRS, N], f16, name="outT_sb")
            ones_sb = persist.tile([33, 64], f16, name="ones_sb")
            warm_row = persist.tile([1, TKB], f16, name="warm_row")
            # rows 0/32 receive each pair's two denominators; rows 1-31 only
            # feed wasted reciprocal lanes but must be nonzero and initialized
            den_q = persist.tile([33, 512], f32, name="den_q")
            with nc.allow_low_precision(reason="ones constant is exact in f16"):
                nc.vector.tensor_copy(
                    v_all[:, :, :, 64:66],
                    nc.const_aps.tensor(1.0, [P, JT, 6, 2], f32),
                )
                nc.vector.tensor_copy(
                    ones_sb[:], nc.const_aps.tensor(1.0, [33, 64], f32)
                )
                nc.vector.tensor_copy(
                    warm_row[:], nc.const_aps.tensor(1.0, [1, TKB], f32)
                )
            nc.vector.tensor_copy(
                den_q[:], nc.const_aps.tensor(1.0, [33, 512], f32)
            )
            # dummy exp forces the ACT table load off the critical path
            warm_sb = persist.tile([1, 64], f16, name="warm_sb")
            nc.scalar.activation(warm_sb[:], ones_sb[0:1, :], Exp)

            apsum = ctx.enter_context(
                tc.tile_pool(name="apsum", bufs=2, space="PSUM")
            )
            spsum = ctx.enter_context(
                tc.tile_pool(name="spsum", bufs=2, space="PSUM")
            )
            opsum = ctx.enter_context(
                tc.tile_pool(name="opsum", bufs=2, space="PSUM")
            )
            expt_pool = ctx.enter_context(tc.tile_pool(name="expt", bufs=6))
            nrm_pool = ctx.enter_context(tc.tile_pool(name="nrm", bufs=2))
            poS_pool = ctx.enter_context(tc.tile_pool(name="poSp", bufs=4))
            outsb_pool = ctx.enter_context(tc.tile_pool(name="outsb", bufs=6))

            # ~6us of dummy matmuls on constants while input DMAs stream:
            # keeps the PE HAM window busy so real matmuls start at 2.4 GHz
            warm_ps = apsum.tile([P, TKB], f32, tag="aps", name="warm_ps")
            for _ in range(12):
                nc.tensor.matmul(
                    warm_ps[0:64, :],
                    ones_sb[0:1, 0:64],
                    warm_row[0:1, :],
                    start=True,
                    stop=True,
                )

            def kq_group(p, tb, which):
                w_sb, dst = (
                    (wk_sb, kT_sb) if which == "k" else (wq_sb, qT_sb)
                )
                ps = apsum.tile([P, TKB], f32, tag="aps")
                for c in range(CT):
                    nc.tensor.matmul(
                        ps[:],
                        w_sb[:, p, c, :],
                        xt_sb[:, tb, c, :],
                        start=(c == 0),
                        stop=(c == CT - 1),
                    )
                with nc.allow_low_precision(reason="f16 kq path"):
                    nc.vector.tensor_copy(dst[:, p, ts(tb, TKB)], ps[:])

            def kq_thunks(p, order="kq"):
                # K first: a pair's scores need all of K but only the
                # current query block of Q
                return [
                    (lambda p=p, tb=tb, w=w: kq_group(p, tb, w))
                    for w in order
                    for tb in range(N // TKB)
                ]

            def v_tile(tt):
                # V projection, all 6 heads at once for one 128-token tile:
                # token-tile stationary, wv moving
                ps = apsum.tile([P, TKB], f32, tag="aps")
                for c in range(CT):
                    nc.tensor.matmul(
                        ps[:, 0:HC],
                        xt_sb[:, tt // 4, c, ts(tt % 4, P)],
                        wv_sb[:, c, :],
                        start=(c == 0),
                        stop=(c == CT - 1),
                    )
                with nc.allow_low_precision(reason="f16 value path"):
                    nc.vector.tensor_copy(
                        v_all[:, tt, :, 0:64],
                        ps[:, 0:HC].rearrange("p (h e) -> p h e", e=64),
                    )

            def out_chunk(git, ci, ob, pp=None, skip_mm=False):
                # half of the final projection for one 128-token tile
                n0, n1 = ((0, 512), (512, 768))[ci]
                if pp is None:
                    pp = apsum.tile([P, TKB], f32, tag="aps")
                if not skip_mm:
                    for t in range(PAIRS):
                        nc.tensor.matmul(
                            pp[:, 0 : n1 - n0],
                            outT_sb[:, t, ds(git * P, P)],
                            wp_sb[:, t, n0:n1],
                            start=(t == 0),
                            stop=(t == PAIRS - 1),
                        )
                with nc.allow_low_precision(reason="f16 partial output"):
                    nc.vector.tensor_add(
                        ob[:, n0:n1], pp[:, 0 : n1 - n0], bias_sb[:, n0:n1]
                    )
                if ci == 1:
                    nc.sync.dma_start(out_r[git], ob[:])

            def og_thunks(gits):
                th = []
                for g in gits:
                    ob_box = []

                    def c0(g=g, ob_box=ob_box):
                        ob_box.append(
                            outsb_pool.tile([P, C], f16, tag="ob", name="ob")
                        )
                        out_chunk(g, 0, ob_box[0])

                    def c1(g=g, ob_box=ob_box):
                        out_chunk(g, 1, ob_box[0])

                    th += [c0, c1]
                return th

            def attention(p, ib, with_v=False, bg=None, fin=None, last=False):
                po0 = opsum.tile([P, 512], f32, tag="po")
                po1 = opsum.tile([P, 512], f32, tag="po")
                pos = (po0, po1)

                def scores(jt):
                    ss = spsum.tile([P, 1024], f32, tag="ss")
                    nc.tensor.matmul(
                        ss[:, 0:512],
                        kT_sb[0:64, p, ts(jt, P)],
                        qT_sb[0:64, p, ts(ib, 512)],
                        start=True,
                        stop=True,
                    )
                    nc.tensor.matmul(
                        ss[:, 512:1024],
                        kT_sb[64:128, p, ts(jt, P)],
                        qT_sb[64:128, p, ts(ib, 512)],
                        start=True,
                        stop=True,
                    )
                    return ss

                ss_cur = scores(0)
                if with_v:
                    v_tile(0)
                for jt in range(JT):
                    et = expt_pool.tile([P, 1024], f16, tag="et")
                    nc.scalar.activation(et[:], ss_cur[:], Exp, scale=SCALE)
                    # next key-tile's scores run on the PE while the exp
                    # above works through this one
                    if jt < JT - 1:
                        ss_cur = scores(jt + 1)
                        if with_v:
                            v_tile(jt + 1)
                    if bg is not None and jt < len(bg) and bg[jt] is not None:
                        bg[jt]()
                    if jt == 6 and fin is not None:
                        fin()
                    for hh in range(2):
                        nc.tensor.matmul(
                            pos[hh][0:65, :],
                            v_all[:, jt, 2 * p + hh, 0:65],
                            et[:, hh * 512 : (hh + 1) * 512],
                            start=(jt == 0),
                            stop=(jt == JT - 1),
                        )
                poS_all = []
                for hh in range(2):
                    poS = poS_pool.tile([65, 512], f32, tag="poS")
                    nc.vector.tensor_copy(poS[:], pos[hh][0:65, :])
                    # stack this head's denominator at partition 32*hh
                    nc.sync.dma_start(
                        den_q[32 * hh : 32 * hh + 1, :], poS[64:65, :]
                    )
                    poS_all.append(poS)
                rd32 = nrm_pool.tile([33, 512], f32, tag="rd32")
                nc.vector.reciprocal_approx_fast(rd32[:], den_q[:])
                rd_q = nrm_pool.tile([33, 512], f16, tag="rd_q")
                with nc.allow_low_precision(reason="f16 reciprocal scale"):
                    nc.vector.tensor_copy(rd_q[:], rd32[:])

                def fin_thunk():
                    rbs = []
                    for hh in range(2):
                        # partition-broadcast 1/den via PE outer product into
                        # a short-lived apsum slot (the po banks are occupied
                        # by live accumulators, which would delay this a
                        # whole block). The final block's fin runs after the
                        # tail prefill has filled every aps/ss slot with work
                        # that depends on this fin's output, so it must use
                        # the po banks instead (free once the casts are done).
                        rb_ps = (opsum if last else apsum).tile(
                            [P, 512], f32, tag="po" if last else "aps"
                        )
                        nc.tensor.matmul(
                            rb_ps[0:64, :],
                            ones_sb[32 * hh : 32 * hh + 1, 0:64],
                            rd_q[32 * hh : 32 * hh + 1, :],
                            start=True,
                            stop=True,
                        )
                        rbs.append(rb_ps)
                    with nc.allow_low_precision(reason="f16 attn output"):
                        if last:
                            # per-git muls so the tail projections can start
                            # on the first token tile while the rest finish
                            for g in range(4):
                                for hh in range(2):
                                    nc.vector.tensor_mul(
                                        outT_sb[
                                            hh * 64 : (hh + 1) * 64,
                                            p,
                                            ds(ib * 512 + g * P, P),
                                        ],
                                        poS_all[hh][0:64, ds(g * P, P)],
                                        rbs[hh][0:64, ds(g * P, P)],
                                    )
                        else:
                            for hh in range(2):
                                nc.vector.tensor_mul(
                                    outT_sb[hh * 64 : (hh + 1) * 64, p, ts(ib, 512)],
                                    poS_all[hh][0:64, :],
                                    rbs[hh][0:64, :],
                                )

                return fin_thunk

            def slots(d, njt=JT):
                out = [None] * njt
                for k, th in d.items():
                    if isinstance(th, list):
                        def chain(ths=th):
                            for t in ths:
                                t()
                        out[k] = chain
                    else:
                        out[k] = th
                return out

            kq0 = kq_thunks(0)   # [k0,k1,k2,k3, q0,q1,q2,q3]
            kq1 = kq_thunks(1)
            kq2 = kq_thunks(2)
            og = og_thunks(range(12))   # 24 chunk thunks for gits 0..11

            kq0[0]()            # k tb0
            kq0[4]()            # q tb0
            # each pair's first block carries that pair's remaining K
            # projections (scores consume kT tile-by-tile); q projections
            # land one block before their query block needs them
            fin = attention(0, 0, with_v=True, bg=slots({
                1: kq0[1], 4: kq0[2], 7: kq0[3], 10: kq0[5],
            }))
            fin = attention(0, 1, bg=slots({
                0: kq0[6], 3: kq1[0], 8: kq1[4], 12: kq1[1],
            }), fin=fin)
            fin = attention(0, 2, bg=slots({
                0: kq0[7], 3: kq1[2], 7: kq1[3], 11: kq1[5],
            }), fin=fin)
            fin = attention(0, 3, bg=slots({
                0: kq1[6], 3: kq2[0], 7: kq2[4], 11: kq2[1],
            }), fin=fin)
            fin = attention(1, 0, bg=slots({
                1: kq2[2], 5: kq2[3], 9: kq1[7], 13: kq2[5],
            }), fin=fin)
            fin = attention(2, 0, bg=slots({
                2: kq2[6], 6: kq2[7],
            }), fin=fin)
            # og(gits of ib) needs outT of all three pairs for that ib: the
            # last fin to land is fin(2, ib), fired at jt6 of the following
            # block, so each og batch spreads over the next two blocks
            fin = attention(1, 1, bg=slots({
                8: og[0], 9: og[1], 11: og[2], 12: og[3],
            }), fin=fin)
            fin = attention(2, 1, bg=slots({
                0: og[4], 2: og[5], 4: og[6], 6: og[7],
            }), fin=fin)
            fin = attention(1, 2, bg=slots({
                8: og[8], 9: og[9], 11: og[10], 12: og[11],
            }), fin=fin)
            fin = attention(2, 2, bg=slots({
                0: og[12], 2: og[13], 4: og[14], 6: og[15],
            }), fin=fin)
            fin = attention(1, 3, bg=slots({
                8: og[16], 9: og[17], 11: og[18], 12: og[19],
            }), fin=fin)
            fin = attention(2, 3, bg=slots({
                0: og[20], 2: og[21], 4: og[22], 6: og[23],
            }), fin=fin, last=True)

            # tail: prefill the pair-0/1 contributions for gits 12-14 while
            # the last exps and the final normalization chain run, then
            # finish with the pair-2 matmuls, bias adds and output DMAs.
            sst0 = spsum.tile([P, 1024], f32, tag="ss", name="sst0")
            sst1 = spsum.tile([P, 1024], f32, tag="ss", name="sst1")
            tail_slots = [
                apsum.tile([P, TKB], f32, tag="aps", name="tp0"),
                apsum.tile([P, TKB], f32, tag="aps", name="tp1"),
                sst0[:, 0:512],
                sst0[:, 512:1024],
                sst1[:, 0:512],
                sst1[:, 512:1024],
            ]
            tail_gc = [(g, ci) for g in range(12, 15) for ci in (0, 1)]
            for k, (g, ci) in enumerate(tail_gc):
                n0, n1 = ((0, 512), (512, 768))[ci]
                pp = tail_slots[k]
                for t in (0, 1):
                    nc.tensor.matmul(
                        pp[:, 0 : n1 - n0],
                        outT_sb[:, t, ds(g * P, P)],
                        wp_sb[:, t, n0:n1],
                        start=(t == 0),
                        stop=False,
                    )
            fin()                                   # outT(2, 3)
            obs = [
                outsb_pool.tile([P, C], f16, tag="ob", name=f"obt{g}")
                for g in range(4)
            ]
            for k, (g, ci) in enumerate(tail_gc):
                n0, n1 = ((0, 512), (512, 768))[ci]
                pp = tail_slots[k]
                nc.tensor.matmul(
                    pp[:, 0 : n1 - n0],
                    outT_sb[:, 2, ds(g * P, P)],
                    wp_sb[:, 2, n0:n1],
                    start=False,
                    stop=True,
                )
                out_chunk(g, ci, obs[g - 12], pp=pp, skip_mm=True)
            # git 15: full projection at the very end
            for ci in (0, 1):
                out_chunk(15, ci, obs[3])

    nc.compile()
    return nc


def _get_nc():
    if "nc" not in _cache:
        _cache["nc"] = _build_bass()
    return _cache["nc"]


def _prep_in_maps(x, w_qkv, w_proj, b_proj):
    x = np.asarray(x, np.float32)
    w_qkv = np.asarray(w_qkv, np.float32)
    w_proj = np.asarray(w_proj, np.float32)
    b_proj = np.asarray(b_proj, np.float32)

    def swz(w):
        # [C_in, M] -> partition-major [128, CT_in * M] (contiguous DMA)
        ct, m = w.shape[0] // P, w.shape[1]
        return np.ascontiguousarray(
            w.reshape(ct, P, m).transpose(1, 0, 2).reshape(P, ct * m)
        ).astype(np.float16)

    def swz_kq(w):
        # [C_in=768, 384] -> pair-major [128, PAIRS * CT * 128]
        return np.ascontiguousarray(
            w.reshape(CT, P, PAIRS, P).transpose(1, 2, 0, 3).reshape(P, -1)
        ).astype(np.float16)

    wq = np.ascontiguousarray(w_qkv[0:C].T)
    wk = np.ascontiguousarray(w_qkv[C : 2 * C].T)
    wv = np.ascontiguousarray(w_qkv[2 * C : 3 * C].T)
    wp = w_proj.T
    bb = np.ascontiguousarray(np.broadcast_to(b_proj[None, :], (P, C)))
    zb = np.zeros((P, C), np.float32)

    in_maps = []
    for core in range(NCORES):
        b, half = core // 2, core % 2
        # x[b].T [C, N] -> [128, TB, CT, 512] partition-major, contiguous
        xt = np.ascontiguousarray(
            x[b].T.reshape(CT, P, N // TKB, TKB)
            .transpose(1, 2, 0, 3)
            .reshape(P, -1)
        ).astype(np.float16)
        sl = slice(half * HC, (half + 1) * HC)
        in_maps.append(
            {
                "xt": xt,
                "wq": swz_kq(wq[:, sl]),
                "wk": swz_kq(wk[:, sl]),
                "wv": swz(wv[:, sl]),
                "wp": swz(wp[sl, :]),
                "bb": bb if half == 0 else zb,
            }
        )
    return in_maps


def run(x, w_qkv, w_proj, b_proj, trace=False):
    from concourse import bass_utils

    nc = _get_nc()
    in_maps = _prep_in_maps(x, w_qkv, w_proj, b_proj)
    br = bass_utils.run_bass_kernel_spmd(
        nc, in_maps, core_ids=list(range(NCORES)), trace=trace
    )
    y = np.empty((B, N, C), np.float32)
    for b in range(B):
        y[b] = np.asarray(br.results[2 * b]["out"], np.float32)
        y[b] += np.asarray(br.results[2 * b + 1]["out"], np.float32)
    return y, br


def kernel(x, w_qkv, w_proj, b_proj):
    y, _ = run(x, w_qkv, w_proj, b_proj, trace=False)
    return y
